# revision 1
# baseline (speedup 1.0000x reference)
"""Trainium2 Bass kernel for nn_DeformRouting (deformable routing conv).

Strategy (8 cores, data-parallel over N x H-halves):
  core c handles image n = c//2, row-half = c%2 (14 rows x 28 cols = 392 pixels).

Per-core device pipeline (points-on-partitions, chunks of 126/126/126/14 pts
so idx columns pack densely: 3x9 full columns + ONE column for the last 14
points with samples laid out p' = pt*9 + kk):
  1. offset conv: 4 PE matmuls into PSUM  out[pt,18] = x_chunk.T @ w_off.T
  2. chunk-3 offsets: permute-matmul replicates the 14-point conv rows to the
     126 p'-slots, then a mask-multiply with free-dim accum selects the
     per-p' kernel-offset channel.
  3. coordinate math (DVE): +64-biased coords (floor via i32 cast with
     round-up correction), one fp16 table index per sample
     idx = clip(y0,63,91)*29 + clip(x0,63,91) - 1890 into a 29x29 2x2-block
     table; bilinear weights w00..w11 with validity folded in.
  4. idx wrap via permutation matmuls (single readiness event per gather
     group), then 5 pipelined dma_gathers (9/9/5/4/1 columns) from the
     841-row fp16 block table (512B rows = 2x2 pixel neighborhood, 64 ch).
  5. combine: samp = w00*A + w01*B + w10*C + w11*D (DVE fp16, contiguous).
  6. PE transposes to m'=(kk,c)-on-partitions; per-chunk accumulating fp16
     matmuls ps1 = W~ @ s, ps2 = B~ @ s; chunk 3's transposed samples are
     scatter-copied into chunk 2's rhs columns and ride its matmuls.
  7. out = x * ps1 + ps2 per chunk, 3 output DMAs.
"""

import numpy as np

import concourse.bass as bass
import concourse.tile as tile
from concourse import bacc, mybir
from concourse.bass_utils import run_bass_kernel_spmd
from concourse.masks import make_identity

# problem constants (hardcoded per contract)
N, CIN, COUT, H, W, K = 4, 64, 64, 28, 28, 3
K2 = K * K  # 9
NCORES = 8
HHALF = H // 2          # 14 rows per core
NPT = HHALF * W         # 392 points per core
CH_ST = [0, 126, 252, 378]    # chunk starts
CH_W = [126, 126, 126, 14]    # chunk widths
M = 3 * K2 + 1          # 28 idx columns (27 regular + 1 for chunk 3)
TBLR = (H + 1) * (W + 1)  # 841 block-table rows
EL = 4 * CIN            # 256 fp16 elems per table row (512B)
SC = (W - 1) / 2.0      # 13.5
BIAS = 64.0             # coordinate bias: keeps floor inputs positive
NBLK = 5                # ceil(576/128) m'-blocks

F32 = mybir.dt.float32
F16 = mybir.dt.float16
I32 = mybir.dt.int32
I16 = mybir.dt.int16

_CACHE = {}


def _alu(name):
    return getattr(mybir.AluOpType, name)


def _build_program():
    """Build + compile the (SPMD-identical) Bass program once."""
    nc = bacc.Bacc("TRN2", target_bir_lowering=False, debug=False,
                   num_devices=NCORES)

    # DRAM I/O (per-core shapes)
    xblk = nc.dram_tensor("xblk", [TBLR, EL], F16, kind="ExternalInput")
    xcpad = nc.dram_tensor("xcpad", [128, NPT], F32, kind="ExternalInput")
    wofft = nc.dram_tensor("wofft", [128, 2 * K2], F32, kind="ExternalInput")
    baseg = nc.dram_tensor("baseg", [128, 2 * M], F32, kind="ExternalInput")
    wwb = nc.dram_tensor("wwb", [128, 10 * COUT], F16, kind="ExternalInput")
    mg = nc.dram_tensor("mg", [128, 8 * 128], F16, kind="ExternalInput")
    permc = nc.dram_tensor("permc", [128, 126], F32, kind="ExternalInput")
    maskxy = nc.dram_tensor("maskxy", [128, 2 * 2 * K2], F32,
                            kind="ExternalInput")
    out_d = nc.dram_tensor("out", [COUT, NPT], F32, kind="ExternalOutput")

    mult, add, sub = _alu("mult"), _alu("add"), _alu("subtract")
    is_eq, is_gt = _alu("is_equal"), _alu("is_gt")
    amin, amax = _alu("min"), _alu("max")

    with tile.TileContext(nc) as tc:
        with (
            tc.tile_pool(name="const", bufs=1) as cpool,
            tc.tile_pool(name="work", bufs=1) as wpool,
            tc.tile_pool(name="psoff", bufs=1, space="PSUM") as opool,
            tc.tile_pool(name="psum", bufs=2, space="PSUM") as ppool,
            tc.tile_pool(name="pso", bufs=1, space="PSUM") as popool,
        ):
            # ---- load inputs; conv-critical ones issue from the scalar
            # engine (free ~2us before sync clears its startup work) ----
            xc_sb = cpool.tile([128, NPT], F32)
            wofft_sb = cpool.tile([128, 2 * K2], F32)
            nc.scalar.dma_start(xc_sb[:, :126], xcpad.ap()[:, :126])
            nc.scalar.dma_start(wofft_sb[:], wofft.ap())
            nc.scalar.dma_start(xc_sb[:, 126:252], xcpad.ap()[:, 126:252])
            nc.sync.dma_start(xc_sb[:, 252:], xcpad.ap()[:, 252:])
            base_sb = cpool.tile([128, 2, M], F32)
            nc.scalar.dma_start(base_sb[:], baseg.ap().rearrange(
                "p (a b) -> p a b", a=2))
            mg_sb = cpool.tile([128, 8, 128], F16)
            nc.sync.dma_start(mg_sb[:], mg.ap().rearrange(
                "p (a b) -> p a b", a=8))
            wwb_sb = cpool.tile([128, 10, COUT], F16)
            nc.sync.dma_start(wwb_sb[:], wwb.ap().rearrange(
                "p (a b) -> p a b", a=10))
            permc_sb = cpool.tile([128, 126], F32)
            nc.sync.dma_start(permc_sb[:], permc.ap())
            mask_sb = cpool.tile([128, 2, 2 * K2], F32)
            nc.sync.dma_start(mask_sb[:], maskxy.ap().rearrange(
                "p (a b) -> p a b", a=2))
            ident = cpool.tile([128, 128], F32)
            make_identity(nc, ident[:])
            ident16 = cpool.tile([128, 128], F16)
            nc.any.tensor_copy(ident16[:], ident[:])

            # ---- 1. offset conv: psum[pt, ch, 18]; pads pre-zeroed so
            # coord math can read the PSUM directly ----
            ps_off = opool.tile([128, 4, 2 * K2], F32)
            nc.vector.memset(ps_off[:], 0.0)
            for ch in range(4):
                st, wd = CH_ST[ch], CH_W[ch]
                nc.tensor.matmul(
                    out=ps_off[:wd, ch, :],
                    lhsT=xc_sb[:, st:st + wd],
                    rhs=wofft_sb[:],
                    start=True, stop=True,
                )

            # ---- 2+3. per-chunk coordinate math (+64-biased) reading the
            # conv PSUM directly, idx -> per-group wrap -> gather; the
            # chunk-3 (col 27) path runs AFTER the main gathers are issued
            # so it never blocks them ----
            _cnt = [0]

            def t(cols=9, dt=F32):
                _cnt[0] += 1
                return wpool.tile([128, cols], dt, name=f"ct{_cnt[0]}")

            idx16 = wpool.tile([128, M], F16, name="idx16")
            ixs = wpool.tile([128, 2, M], F32, name="ixs")
            f0s = wpool.tile([128, 2, M], F32, name="f0s")
            w4 = [wpool.tile([128, M], F16, name=f"w4_{i}")
                  for i in range(4)]

            def coord_part(off_ap, cols, csl):
                """coords + table idx for columns csl (writes ixs/f0s/idx16).

                f0 = floor(i): the f32->i32 cast rounds-to-nearest, so
                subtract 1 where it rounded up."""
                for axis in range(2):
                    i_c = ixs[:, axis, csl]
                    nc.vector.scalar_tensor_tensor(
                        i_c, off_ap(axis), SC, base_sb[:, axis, csl],
                        mult, add)
                    ti = t(cols, I32)
                    nc.vector.tensor_copy(ti[:], i_c)
                    tf = t(cols)
                    nc.vector.tensor_copy(tf[:], ti[:])
                    g = t(cols)
                    nc.vector.tensor_tensor(g[:], tf[:], i_c, is_gt)
                    nc.vector.tensor_tensor(f0s[:, axis, csl], tf[:],
                                            g[:], sub)
                xb = t(cols)
                nc.vector.tensor_scalar(xb[:], f0s[:, 0, csl], 91.0, 63.0,
                                        amin, amax)
                yb = t(cols)
                nc.vector.tensor_scalar(yb[:], f0s[:, 1, csl], 91.0, 63.0,
                                        amin, amax)
                idx_f = t(cols)
                nc.vector.scalar_tensor_tensor(idx_f[:], yb[:], 29.0,
                                               xb[:], mult, add)
                nc.vector.tensor_scalar_add(idx16[:, csl], idx_f[:],
                                            -1890.0)

            def wrap_grp(wi, lo, ncol):
                # wrap[q, m*8+g] = idx16[g*16 + q%16, m]. Group 0's copy
                # runs on gpsimd so the first gather follows it in-queue
                # with no cross-engine semaphore hop.
                psw = opool.tile([128, 8, 9], F32, tag="psw",
                                 name=f"psw{wi}")
                for gsel in range(8):
                    nc.tensor.matmul(
                        out=psw[:, gsel, :ncol], lhsT=mg_sb[:, gsel, :],
                        rhs=idx16[:, lo:lo + ncol], start=True, stop=True)
                nc.any.tensor_copy(
                    wrap[:, lo:lo + ncol, :].rearrange("q m g -> q g m"),
                    psw[:, :, :ncol])

            wrap = wpool.tile([128, M, 8], I16, name="wrap")
            for ch in range(3):
                coord_part(lambda axis: ps_off[:, ch, axis:18:2],
                           9, slice(ch * K2, (ch + 1) * K2))
                wrap_grp(ch, ch * K2, K2)

            # gather segments: (chunk, col_lo, ncols); chunk 2 is split so
            # its exposed transfer+combine tail is short; chunk 3 is one
            # 128-slot column
            SEGS = {0: [(0, 0, K2)], 1: [(1, K2, K2)],
                    2: [(2, 18, 6), (2, 24, 3)], 3: [(3, 27, 1)]}
            gq = {}
            for segs in SEGS.values():
                for ch, clo, ncol in segs:
                    gq[(ch, clo)] = wpool.tile([128, ncol, EL], F16,
                                               name=f"g{ch}_{clo}")

            def gather_seg(ch, clo, ncol):
                ni = 128 * ncol
                nc.gpsimd.dma_gather(
                    out_ap=gq[(ch, clo)][:],
                    in_ap=xblk.ap(),
                    idxs_ap=wrap[:, clo:clo + ncol, :].rearrange(
                        "q m g -> q (m g)"),
                    num_idxs=ni, num_idxs_reg=ni, elem_size=EL,
                    single_packet=False)

            for ch in range(3):
                for ch_, clo, ncol in SEGS[ch]:
                    gather_seg(ch_, clo, ncol)

            # chunk-3 offsets -> p' = pt*9+kk layout (col 27): replicate
            # the 14 conv rows to 126 p'-slots via a permutation matmul,
            # then mask-multiply + free-dim accum selects the 2*kk(p')
            # (+axis) offset channel
            sb3 = wpool.tile([128, 2 * K2], F32, name="sb3")
            nc.any.tensor_copy(sb3[:14, :], ps_off[:14, 3, :])
            ps36 = opool.tile([128, 2 * K2], F32, name="ps36")
            nc.tensor.matmul(out=ps36[:126, :], lhsT=permc_sb[:14, :126],
                             rhs=sb3[:14, :], start=True, stop=True)
            off27 = wpool.tile([128, 2], F32, name="off27")
            nc.vector.memset(off27[:], 0.0)
            junk = wpool.tile([128, 2 * K2], F32, name="junk")
            for axis in range(2):
                nc.vector.scalar_tensor_tensor(
                    junk[:126, :], ps36[:126, :], 1.0,
                    mask_sb[:126, axis, :], mult, mult,
                    accum_out=off27[:126, axis:axis + 1])
            coord_part(lambda axis: off27[:, axis:axis + 1],
                       1, slice(27, 28))
            wrap_grp(3, 27, 1)
            gather_seg(3, 27, 1)

            # ---- weights, one pass over all 28 cols (overlaps desc-gen);
            # wquad = per-corner col-27 weights broadcast over channels for
            # the 3-op chunk-3 combine ----
            def frac(axis):
                w1 = t(M)
                nc.vector.tensor_tensor(w1[:], ixs[:, axis, :],
                                        f0s[:, axis, :], sub)
                w0 = t(M)
                nc.vector.tensor_scalar(w0[:], w1[:], -1.0, 1.0, mult, add)
                return w0, w1

            def valid01(axis):
                f0 = f0s[:, axis, :]
                c0 = t(M)
                nc.vector.tensor_scalar(c0[:], f0, 91.0, 64.0, amin, amax)
                v0 = t(M)
                nc.vector.tensor_tensor(v0[:], c0[:], f0, is_eq)
                f1 = t(M)
                nc.vector.tensor_scalar_add(f1[:], f0, 1.0)
                c1 = t(M)
                nc.vector.tensor_scalar(c1[:], f1[:], 91.0, 64.0,
                                        amin, amax)
                v1 = t(M)
                nc.vector.tensor_tensor(v1[:], c1[:], f1[:], is_eq)
                return v0, v1

            wx0, wx1 = frac(0)
            wy0, wy1 = frac(1)
            vx0, vx1 = valid01(0)
            vy0, vy1 = valid01(1)

            def vmul(a, b, out=None):
                if out is None:
                    o = t(M)
                    nc.vector.tensor_tensor(o[:], a[:], b[:], mult)
                    return o
                nc.vector.tensor_tensor(out, a[:], b[:], mult)

            wx0v, wx1v = vmul(wx0, vx0), vmul(wx1, vx1)
            wy0v, wy1v = vmul(wy0, vy0), vmul(wy1, vy1)
            vmul(wy0v, wx0v, out=w4[0][:])
            vmul(wy0v, wx1v, out=w4[1][:])
            vmul(wy1v, wx0v, out=w4[2][:])
            vmul(wy1v, wx1v, out=w4[3][:])
            wquad = wpool.tile([128, 4 * CIN], F16, name="wquad")
            for xy in range(4):
                nc.any.tensor_copy(
                    wquad[:, xy * CIN:(xy + 1) * CIN],
                    w4[xy][:, 27:28].to_broadcast([128, CIN]))

            # ---- 5+6. per-chunk combine (fp16, independent muls + tree
            # adds), PE transposes, per-chunk matmuls + fold + out DMA ----
            samp = wpool.tile([128, 3, K2, CIN], F16, name="samp")
            samp3 = wpool.tile([128, 1, CIN], F16, name="samp3")
            tm = [wpool.tile([128, K2, CIN], F16, name=f"tm{i}")
                  for i in range(3)]
            rhs = wpool.tile([128, NBLK, NPT], F16, name="rhs")
            # rows 64:128 of the last m'-block are padding (576 -> 640): the
            # K=128 matmul reads them, so they must be zeroed. Chunk 3's rhs
            # columns are only partially covered by its 9 scatter copies, so
            # zero them up front too.
            nc.any.memset(rhs[64:, NBLK - 1, :], 0.0)
            nc.any.memset(rhs[:64, :, 378:], 0.0)
            nc.any.memset(rhs[64:, :NBLK - 1, 378:], 0.0)
            ps1 = popool.tile([COUT, NPT], F32, name="ps1")
            ps2 = popool.tile([COUT, NPT], F32, name="ps2")
            out_sb = wpool.tile([COUT, NPT], F32)

            for ch in range(4):
                kdone = 0
                bdone = 0
                for ch_, clo, ncol in SEGS[ch]:
                    g = gq[(ch_, clo)]
                    qsl = slice(clo, clo + ncol)

                    def bc(wt):
                        return wt[:, qsl, None].to_broadcast(
                            [128, ncol, CIN])

                    if ch == 3:
                        # 3-op combine: one wquad multiply, then a 2-level
                        # slice-add reduction over the 4 corners
                        prod = tm[0][:].rearrange("p k c -> p (k c)")
                        nc.vector.tensor_tensor(
                            prod[:, :4 * CIN],
                            g[:].rearrange("p a c -> p (a c)"),
                            wquad[:], mult)
                        th = tm[1][:].rearrange("p k c -> p (k c)")
                        nc.vector.tensor_tensor(
                            th[:, :2 * CIN], prod[:, :2 * CIN],
                            prod[:, 2 * CIN:4 * CIN], add)
                        nc.vector.tensor_tensor(
                            samp3[:, 0, :], th[:, 0:CIN],
                            th[:, CIN:2 * CIN], add)
                        continue
                    klo = clo - ch * K2
                    sv = samp[:, ch, klo:klo + ncol]
                    t0 = tm[0][:, :ncol]
                    t1 = tm[1][:, :ncol]
                    t2 = tm[2][:, :ncol]
                    nc.vector.tensor_tensor(sv, g[:, :, 0:CIN],
                                            bc(w4[0]), mult)
                    nc.vector.tensor_tensor(t0, g[:, :, CIN:2 * CIN],
                                            bc(w4[1]), mult)
                    nc.vector.tensor_tensor(t1, g[:, :, 2 * CIN:3 * CIN],
                                            bc(w4[2]), mult)
                    nc.vector.tensor_tensor(t2, g[:, :, 3 * CIN:],
                                            bc(w4[3]), mult)
                    nc.vector.tensor_tensor(sv, sv, t0, add)
                    nc.vector.tensor_tensor(t1, t1, t2, add)
                    nc.vector.tensor_tensor(sv, sv, t1, add)
                    # emit transpose blocks fully covered by combined cols
                    kdone += ncol
                    svf = samp[:, ch].rearrange("p k c -> p (k c)")
                    while bdone < NBLK:
                        mlo = 128 * bdone
                        mhi = min(128 * (bdone + 1), K2 * CIN)
                        if mhi > kdone * CIN:
                            break
                        pst = ppool.tile([128, 128], F16, tag="tps")
                        nc.tensor.transpose(
                            pst[:mhi - mlo, :], svf[:, mlo:mhi], ident16[:])
                        nc.any.tensor_copy(
                            rhs[:mhi - mlo, bdone,
                                CH_ST[ch]:CH_ST[ch] + 126],
                            pst[:mhi - mlo, :126])
                        bdone += 1

                if ch == 3:
                    # transpose samp3 -> [c, p'=(pt,kk)], then scatter the
                    # kk-strided columns into this chunk's rhs columns;
                    # its matmul/fold/DMA ride along with chunk 2's below
                    pst3 = ppool.tile([128, 128], F16, tag="tps")
                    nc.tensor.transpose(
                        pst3[:CIN, :],
                        samp3[:].rearrange("p a c -> p (a c)"), ident16[:])
                    for kk in range(K2):
                        dst = rhs[(kk % 2) * 64:(kk % 2) * 64 + 64,
                                  kk // 2, 378:392]
                        src3 = pst3[:CIN, kk:126:K2]
                        if kk % 2 == 0:
                            nc.vector.tensor_copy(dst, src3)
                        else:
                            nc.scalar.copy(dst, src3)
                if ch == 2:
                    continue  # chunk 2 is emitted together with chunk 3
                st = CH_ST[min(ch, 2)]
                wd = 126 if ch < 2 else 140
                csl = slice(st, st + wd)
                for b in range(NBLK):
                    nc.tensor.matmul(
                        out=ps1[:, csl], lhsT=wwb_sb[:, b, :],
                        rhs=rhs[:, b, csl],
                        start=(b == 0), stop=(b == NBLK - 1))
                    nc.tensor.matmul(
                        out=ps2[:, csl], lhsT=wwb_sb[:, NBLK + b, :],
                        rhs=rhs[:, b, csl],
                        start=(b == 0), stop=(b == NBLK - 1))
                nc.vector.tensor_tensor(out_sb[:, csl], ps1[:, csl],
                                        xc_sb[:COUT, csl], mult)
                nc.vector.tensor_tensor(out_sb[:, csl], out_sb[:, csl],
                                        ps2[:, csl], add)
                nc.sync.dma_start(out_d.ap()[:, csl], out_sb[:, csl])

    nc.compile()
    return nc


def _host_inputs(x, w_off, b_off, w_wgt, b_wgt):
    """Build the 8 per-core input dicts (layout/shard prep only)."""
    x = np.asarray(x, dtype=np.float32)
    w_off = np.asarray(w_off, dtype=np.float32)
    b_off = np.asarray(b_off, dtype=np.float32)
    w_wgt = np.asarray(w_wgt, dtype=np.float32)
    b_wgt = np.asarray(b_wgt, dtype=np.float32)

    xs = np.linspace(-1.0, 1.0, W).astype(np.float32)
    ys = np.linspace(-1.0, 1.0, H).astype(np.float32)
    kx = np.linspace(-(K - 1) / (W - 1), (K - 1) / (W - 1), K).astype(np.float32)
    ky = np.linspace(-(K - 1) / (H - 1), (K - 1) / (H - 1), K).astype(np.float32)

    # wwb [128, 10, 64] fp16: blocks 0..4 = W~ rows m'=(kk*64+c2), blocks
    # 5..9 = B~, where W~[m', o] = w_wgt[o, c2*9+kk], B~[m', o] =
    # b_wgt[o*576 + c2*9 + kk]  (row-permuted from the (c2*9+kk) order).
    perm = np.arange(K2 * CIN).reshape(CIN, K2).T.reshape(-1)  # m' -> c2*9+kk
    wtp = np.zeros((640, COUT), dtype=np.float32)
    wtp[:576] = w_wgt.T[perm]
    btp = np.zeros((640, COUT), dtype=np.float32)
    btp[:576] = b_wgt.reshape(COUT, K2 * CIN).T[perm]
    wwb = np.concatenate([wtp.reshape(5, 128, COUT),
                          btp.reshape(5, 128, COUT)], axis=0)  # [10,128,64]
    wwb = wwb.transpose(1, 0, 2).reshape(128, 10 * COUT).astype(np.float16)

    # idx-wrap permutation selectors: mg[pt, g*128+q] = (pt == g*16 + q%16)
    mg = np.zeros((128, 8, 128), dtype=np.float16)
    q = np.arange(128)
    for gsel in range(8):
        mg[gsel * 16 + (q % 16), gsel, q] = 1.0
    mg = mg.reshape(128, 8 * 128)

    wofft = np.zeros((128, 2 * K2), dtype=np.float32)
    wofft[:CIN] = w_off.T

    # chunk-3 p'-replication matrix and offset-channel selection masks
    pidx = np.arange(126)
    permc = np.zeros((128, 126), dtype=np.float32)
    permc[pidx // K2, pidx] = 1.0
    maskxy = np.zeros((128, 2, 2 * K2), dtype=np.float32)
    maskxy[pidx, 0, 2 * (pidx % K2)] = 1.0
    maskxy[pidx, 1, 2 * (pidx % K2) + 1] = 1.0
    maskxy = maskxy.reshape(128, 4 * K2)

    in_maps = []
    for c in range(NCORES):
        n, half = divmod(c, 2)
        r0 = HHALF * half
        xn = x[n]                             # [64, 28, 28]
        x_hwc = xn.transpose(1, 2, 0)         # [28, 28, 64]

        # 2x2 block table: zero-pad image by 1 on all sides, row (yb, xb)
        # holds the 4 corners of the block at (yb-1, xb-1).
        pad = np.zeros((H + 2, W + 2, CIN), dtype=np.float32)
        pad[1:H + 1, 1:W + 1] = x_hwc
        tbl = np.concatenate([
            pad[:H + 1, :W + 1], pad[:H + 1, 1:W + 2],
            pad[1:H + 2, :W + 1], pad[1:H + 2, 1:W + 2],
        ], axis=2).reshape(TBLR, EL).astype(np.float16)

        xcpad = np.zeros((128, NPT), dtype=np.float32)
        xcpad[:CIN] = xn.reshape(CIN, H * W)[:, r0 * W:r0 * W + NPT]

        # base grids [128, 2, 28], +64-biased
        bg = np.full((128, 2, M), SC + BIAS, dtype=np.float32)
        for ch in range(3):
            p_idx = np.arange(126)
            gpix = r0 * W + CH_ST[ch] + p_idx      # global pixel
            row, col = gpix // W, gpix % W
            for kk in range(K2):
                kyi, kxi = divmod(kk, K)
                m = ch * K2 + kk
                bg[:126, 0, m] = (xs[col] + kx[kxi] + b_off[2 * kk]
                                  + 1.0) * SC + BIAS
                bg[:126, 1, m] = (ys[row] + ky[kyi] + b_off[2 * kk + 1]
                                  + 1.0) * SC + BIAS
        # col 27: p' = pt_local*9 + kk for the last 14 points
        pp = np.arange(126)
        gpix = r0 * W + 378 + pp // K2
        row, col = gpix // W, gpix % W
        kk = pp % K2
        kyi, kxi = kk // K, kk % K
        bg[:126, 0, 27] = (xs[col] + kx[kxi] + b_off[2 * kk]
                           + 1.0) * SC + BIAS
        bg[:126, 1, 27] = (ys[row] + ky[kyi] + b_off[2 * kk + 1]
                           + 1.0) * SC + BIAS

        in_maps.append({
            "xblk": tbl,
            "xcpad": xcpad,
            "wofft": wofft,
            "baseg": bg.reshape(128, 2 * M),
            "wwb": wwb,
            "mg": mg,
            "permc": permc,
            "maskxy": maskxy,
        })
    return in_maps


def get_program():
    if "nc" not in _CACHE:
        _CACHE["nc"] = _build_program()
    return _CACHE["nc"]


def run_cores(in_maps, **kw):
    nc = get_program()
    return run_bass_kernel_spmd(nc, in_maps, core_ids=list(range(NCORES)), **kw)


def assemble(results):
    out = np.zeros((N, COUT, H, W), dtype=np.float32)
    for c in range(NCORES):
        n, half = divmod(c, 2)
        out[n, :, HHALF * half:HHALF * (half + 1), :] = \
            results[c]["out"].reshape(COUT, HHALF, W)
    return out


def kernel(x, w_off, b_off, w_wgt, b_wgt):
    in_maps = _host_inputs(x, w_off, b_off, w_wgt, b_wgt)
    res = run_cores(in_maps)
    return assemble(res.results)

